# revision 1
# baseline (speedup 1.0000x reference)
"""Trainium2 Bass kernel for nn_Attention (dense_transformer, ridge regime).

Computation per batch b:
    scores[s]  = <lstm_output[b,s,:], hidden[b,:]>          # [S]
    w          = softmax(scores)                            # [S]
    attn[h]    = sum_s w[s] * lstm_output[b,s,h]            # [H]
    out[b]     = [hidden[b], attn] @ W_combine.T + b_combine

Sharding: data-parallel over batch B=64 across 8 cores (8 batches/core).
W_combine is passed host-transposed (W.T, [2H, H]) and replicated.

Per-core dataflow (all engines explicit, raw bass, one sem wait per instr):
  - DMA: hidden, bias, then L(b) [128,16,1024] per batch (double-buffered),
    then W.T chunks, then the output.
  - PE: replicates hidden[b] across 128 partitions (f32 ones-matmul into
    PSUM), does the weighted sum (einsum2) and final projection as f32r
    matmuls, plus tiny transposes/broadcasts for the softmax reductions.
  - DVE: fused multiply+reduce (tensor_tensor_reduce) computes scores
    directly from the natural [s_partition, h_free] layout -- no transpose
    of the big tensor anywhere.
  - ACT: exp (with -max bias and accumulated partition sums) and the
    normalization-fused copies.
"""

import numpy as np

import concourse.bass as bass
from concourse import bass_isa, library_config, mybir
from concourse.bass_utils import run_bass_kernel_spmd

F32 = mybir.dt.float32
F32R = mybir.dt.float32r

B, S, H = 64, 2048, 1024
NCORES = 8
BPC = B // NCORES          # batches per core
T = S // 128               # s-tiles per batch
NCH = (2 * H) // 128       # 16 chunks of the combined dim
HCH = H // 128             # 8 chunks of one H

_cached_nc = None
last_results = None
PHASE = 6   # debug (55=attnT+ctcp):
SUBV = 0   # phase-5 subvariant 1=scores 2=+maxchain 3=+exp/Z 4=+einsum2/attncopy 5=+attnT/ct 6=full


def _build_program():
    nc = bass.Bass()

    lstm_d = nc.declare_dram_parameter("lstm_output", [BPC, S, H], F32, isOutput=False)
    hid_d = nc.declare_dram_parameter("hidden", [BPC, H], F32, isOutput=False)
    wt_d = nc.declare_dram_parameter("w_t", [2 * H, H], F32, isOutput=False)
    b_d = nc.declare_dram_parameter("b_combine", [H], F32, isOutput=False)
    out_d = nc.declare_dram_parameter("out", [BPC, H], F32, isOutput=True)

    # ---- SBUF ----
    L = [nc.alloc_sbuf_tensor(f"L{i}", [128, T, H], F32R) for i in range(2)]  # 2x8MB
    # W.T reuses L slot 0 at the tail (its DMA starts once einsum2(BPC-2) done)
    WT = L[0]
    hid_t = nc.alloc_sbuf_tensor("hid", [BPC, H], F32)
    hid = hid_t.ap()
    bias_t = nc.alloc_sbuf_tensor("bias", [BPC, H], F32)
    bias = bias_t.ap()
    out_t = nc.alloc_sbuf_tensor("out_sb", [BPC, H], F32)
    out_sb = out_t.ap()
    prod = [nc.alloc_sbuf_tensor(f"prod{i}", [128, H], F32) for i in range(4)]
    dmy = nc.alloc_sbuf_tensor("dmy", [128, T], F32)
    hidR = nc.alloc_sbuf_tensor("hidR", [128, BPC, H], F32)   # 4MB bcast hidden
    CT = nc.alloc_sbuf_tensor("CT", [128, NCH, BPC], F32R)                   # combined^T
    scores = [nc.alloc_sbuf_tensor(f"scores{i}", [128, T], F32) for i in range(2)]
    wexp = [nc.alloc_sbuf_tensor(f"wexp{i}", [128, T], F32R) for i in range(2)]
    zp = [nc.alloc_sbuf_tensor(f"zp{i}", [128, 1], F32) for i in range(2)]
    mp = nc.alloc_sbuf_tensor("mp", [128, 1], F32)
    negM1_t = nc.alloc_sbuf_tensor("negM1s", [1, 2], F32)
    negM1 = [negM1_t.ap()[0:1, i:i + 1] for i in range(2)]
    negM = [nc.alloc_sbuf_tensor(f"negM{i}", [128, 1], F32) for i in range(2)]
    rZ_t = nc.alloc_sbuf_tensor("rZs", [1, 2], F32)
    rZ = [rZ_t.ap()[0:1, i:i + 1] for i in range(2)]
    ones128 = nc.alloc_sbuf_tensor("ones128", [128, 1], F32)
    attn2 = nc.alloc_sbuf_tensor("attn2", [1, 2 * H], F32)
    attn_sb = [attn2.ap()[0:1, i * H:(i + 1) * H] for i in range(2)]
    ones_col = nc.alloc_sbuf_tensor("ones_col", [1, 128], F32)
    ident = nc.alloc_sbuf_tensor("ident", [128, 128], F32)
    sel = nc.alloc_sbuf_tensor("sel", [BPC, BPC, 128], F32)  # sel[k,b,:]=(k==b)

    # ---- PSUM: one bank per concurrent PE write target (the PE wedges on
    # concurrent matmul/transpose-group writes sharing a bank) ----
    acc_lo = nc.alloc_psum_tensor("acc_lo", [BPC, 512], F32)  # einsum2 row 0 / final
    acc_hi = nc.alloc_psum_tensor("acc_hi", [BPC, 512], F32)
    ct8_t = nc.alloc_psum_tensor("ct8", [128, HCH, BPC], F32) # setup transposes
    ctc_t = nc.alloc_psum_tensor("ctc", [128, 512], F32)      # attnT transposes
    stage = nc.alloc_psum_tensor("stage", [128, 512], F32)    # hidR staging mms
    mpT_t = nc.alloc_psum_tensor("mpT", [1, 128], F32)        # transp target
    negM_t = nc.alloc_psum_tensor("negMbc", [128, 1], F32)    # bcast mm target
    Zps_t = nc.alloc_psum_tensor("Zps", [1, 1], F32)          # Z mm target
    mpT = mpT_t.ap()
    negM_bc = negM_t.ap()
    Zps = Zps_t.ap()
    ctcols8 = ct8_t.ap()
    ctcols = ctc_t.ap()[:, 0:HCH]
    stage2 = ctc_t.ap()   # startup-only reuse of the attnT bank

    # ---------------- two-pass emission ----------------
    # ev: event-key -> (sem_name, value).  sems: sem_name -> handle (pass 2).
    ev = {}
    sems = {}
    counts = {}

    class Prog:
        def __init__(self, name):
            self.name = name
            self.emit = False
            self.eng = None
            self.hwm = {}
            # strict-FIFO engines still need pipeline drains between
            # dependent ops for well-defined same-engine ordering
            self.auto_drain = name in ("dve", "act", "gps")
            self.first_op = True

        def begin(self, eng=None, emit=False):
            self.emit = emit
            self.eng = eng
            self.hwm = {}
            self.first_op = True

        def wait(self, key):
            """key: event tuple, or (sem_name, value) pair."""
            if len(key) == 2 and isinstance(key[1], int) and key[0] in (
                    "pe", "dve", "act", "gps", "hid", "bias", "l0", "l1",
                    "wt", "outd", "gdma", "q0", "q1", "q2", "q3"):
                sname, val = key
            else:
                if self.emit and key not in ev:
                    raise KeyError(f"wait on unknown event {key}")
                sname, val = ev.get(key, (None, 0))
            if val <= 0 or sname is None:
                return
            if self.hwm.get(sname, -1) >= val:
                return
            self.hwm[sname] = val
            if self.emit:
                self.eng.wait_ge(sems[sname], val)

        def op(self, fn, inc=1, sem=None, drain=None):
            sname = sem or self.name
            counts[sname] = counts.get(sname, 0) + inc
            if self.emit:
                do_drain = self.auto_drain if drain is None else drain
                if do_drain and not self.first_op:
                    self.eng.drain()
                inst = fn()
                inst.then_inc(sems[sname], inc)
            self.first_op = False

        def mark(self, *key, sem=None):
            sname = sem or self.name
            ev[(self.name,) + tuple(key)] = (sname, counts.get(sname, 0))

    DMA, PE, DVE, ACT, GPS = Prog("dma"), Prog("pe"), Prog("dve"), Prog("act"), Prog("gps")

    bias_src = b_d[:]
    bias_bcast = bass.AP(
        tensor=bias_src.tensor,
        offset=bias_src.offset,
        ap=[[0, BPC]] + list(bias_src.ap),
    )

    def prog_gps():
        g = GPS.eng if GPS.emit else None
        GPS.op(lambda: g.memset(ones_col.ap(), 1.0))
        GPS.op(lambda: g.memset(ones128.ap(), 1.0))
        GPS.op(lambda: g.memset(ident.ap(), 0.0))
        GPS.op(lambda: g.affine_select(
            out=ident.ap(), in_=ident.ap(),
            compare_op=mybir.AluOpType.not_equal, fill=1.0, base=0,
            pattern=[[-1, 128]], channel_multiplier=1))
        GPS.op(lambda: g.memset(sel.ap(), 0.0), drain=True)
        GPS.op(lambda: g.affine_select(
            out=sel.ap(), in_=sel.ap(),
            compare_op=mybir.AluOpType.not_equal, fill=1.0, base=0,
            pattern=[[-1, BPC], [0, 128]], channel_multiplier=1), drain=True)
        GPS.mark("setup")

    def prog_dma():
        d = DMA.eng if DMA.emit else None
        DMA.op(lambda: d.dma_start(out=hid, in_=hid_d[:]), inc=16, sem="hid")
        DMA.mark("hid", sem="hid")
        DMA.op(lambda: d.dma_start(out=bias, in_=bias_bcast), inc=16, sem="bias")
        DMA.mark("bias", sem="bias")
        for b in range(BPC):
            if b >= 2:
                if PHASE >= 4:
                    DMA.wait(("pe", "e2", b - 2))
                else:
                    DMA.wait(("dve", "mult", b - 2, T - 1))
            src = lstm_d[b].rearrange("(t p) h -> p t h", p=128).bitcast(F32R)
            if b == 0:
                # batch 0 arrives in quarters: the first multiplies start
                # ~4x sooner than waiting on the whole 8MB transfer
                for q in range(4):
                    DMA.op(lambda src=src, q=q: d.dma_start(
                        out=L[0].ap()[:, 4 * q:4 * (q + 1), :],
                        in_=src[:, 4 * q:4 * (q + 1), :]),
                        inc=16, sem=f"q{q}")
                    DMA.mark("Lq", q, sem=f"q{q}")
                DMA.mark("L", 0, sem="q3")
            else:
                DMA.op(lambda src=src, b=b: d.dma_start(
                    out=L[b % 2].ap(), in_=src), inc=16, sem=f"l{b % 2}")
                DMA.mark("L", b, sem=f"l{b % 2}")
        if PHASE >= 6:
            DMA.wait(("pe", "e2", BPC - 2))
            wt_src = wt_d[:].rearrange("(c p) n -> p c n", p=128).bitcast(F32R)
            DMA.op(lambda: d.dma_start(out=WT.ap(), in_=wt_src), inc=16, sem="wt")
            DMA.mark("wt", sem="wt")
            DMA.wait(("dve", "bias_hi"))
            out_src = out_sb
        if True:
            pass
        if PHASE < 6:
            gate = {1: ("act", "acc", BPC - 1, T - 1),
                    2: ("dve", "rmax2", BPC - 1),
                    3: ("act", "exp", BPC - 1),
                    4: ("act", "cphi", BPC - 1),
                    5: ("pe", "attnT", BPC - 1),
                    55: ("act", "ctcp", BPC - 1)}[PHASE]
            DMA.wait(gate)
        out_src = out_sb if PHASE >= 6 else hid
        DMA.op(lambda: d.dma_start(out=out_d[:], in_=out_src), inc=16, sem="outd")
        DMA.wait(("outd", counts.get("outd", 0)))

    def prog_pe():
        p = PE.eng if PE.emit else None
        PE.wait(("gps", "setup"))
        PE.wait(("dma", "hid"))
        # hidden^T -> CT chunks 0..7 staging (psum)
        for c in range(HCH):
            PE.op(lambda c=c: p.transpose(
                ctcols8[:, c, :], hid[0:BPC, c * 128:(c + 1) * 128],
                ident.ap()[0:BPC, 0:BPC]))
        PE.mark("hidT")
        # replicate hidden rows across partitions: sel-matmul into the
        # staging bank, DVE/ACT copy out to hidR (all before L(0) lands)
        for k in range(2 * BPC):
            b, j = divmod(k, 2)
            if k == 1:
                PE.wait(("dve", "cth"))   # ctc bank free of setup readers
            if k > 1:
                # wait for the same-bank stage copy two steps back
                pb, pj = divmod(k - 2, 2)
                PE.wait(("dve" if k % 2 == 0 else "act", "hcp", pb, pj))
            tgt = stage.ap() if k % 2 == 0 else stage2
            PE.op(lambda b=b, j=j, tgt=tgt: p.matmul(
                tgt, lhsT=sel.ap()[:, b, :],
                rhs=hid[0:BPC, j * 512:(j + 1) * 512],
                start=True, stop=True))
            PE.mark("hmm", b, j)
        if PHASE >= 2:
            PE.wait(("dve", "rmax", 0))
            PE.op(lambda: p.transpose(mpT, mp.ap(), ident.ap()))
            PE.mark("transp", 0)
        for b in range(BPC):
            if PHASE >= 2:
                PE.wait(("dve", "rmax2", b))
                PE.op(lambda b=b: p.matmul(
                    negM_bc, lhsT=ones_col.ap(), rhs=negM1[b % 2],
                    start=True, stop=True))
                PE.mark("bcast", b)
            if PHASE >= 3:
                if b >= 1:
                    PE.wait(("dve", "recip", b - 1))
                PE.wait(("act", "exp", b))
                PE.op(lambda b=b: p.matmul(
                    Zps, lhsT=zp[b % 2].ap(), rhs=ones128.ap(),
                    start=True, stop=True))
                PE.mark("z", b)
            if PHASE >= 4:
                # einsum2: attn_unnorm = sum_s w[s] * L[s, :]
                PE.wait(("act", "exp", b))
                if b >= 1:
                    PE.wait(("act", "cphi", b - 1))
                for t in range(T):
                    PE.op(lambda b=b, t=t: p.matmul(
                        acc_lo.ap()[0:1, :],
                        lhsT=wexp[b % 2].ap()[:, t:t + 1],
                        rhs=L[b % 2].ap()[:, t, 0:512],
                        start=(t == 0), stop=(t == T - 1)))
                    PE.op(lambda b=b, t=t: p.matmul(
                        acc_hi.ap()[0:1, :],
                        lhsT=wexp[b % 2].ap()[:, t:t + 1],
                        rhs=L[b % 2].ap()[:, t, 512:1024],
                        start=(t == 0), stop=(t == T - 1)))
                PE.mark("e2", b)
            if b + 1 < BPC and PHASE >= 2:
                PE.wait(("dve", "rmax", b + 1))
                PE.op(lambda: p.transpose(mpT, mp.ap(), ident.ap()))
                PE.mark("transp", b + 1)
            if PHASE >= 5:
                # attn row -> columns (chunk transposes via K=1 matmuls)
                PE.wait(("act", "cphi", b))
                if b >= 1 and PHASE in (55, 6):
                    PE.wait(("act", "ctcp", b - 1))
                for c in range(HCH):
                    PE.op(lambda b=b, c=c: p.transpose(
                        ctcols[:, c:c + 1],
                        attn_sb[b % 2][0:1, c * 128:(c + 1) * 128],
                        ones_col.ap()[0:1, 0:1]))
                PE.mark("attnT", b)
        # final projection
        if PHASE < 6:
            return
        PE.wait(("act", "ctcp", BPC - 1))
        PE.wait(("dma", "wt"))
        for c in range(NCH):
            PE.op(lambda c=c: p.matmul(
                acc_lo.ap()[0:BPC, :],
                lhsT=CT.ap()[:, c, :],
                rhs=WT.ap()[:, c, 0:512],
                start=(c == 0), stop=(c == NCH - 1)))
            PE.op(lambda c=c: p.matmul(
                acc_hi.ap()[0:BPC, :],
                lhsT=CT.ap()[:, c, :],
                rhs=WT.ap()[:, c, 512:1024],
                start=(c == 0), stop=(c == NCH - 1)))
        PE.mark("final")

    def prog_dve():
        v = DVE.eng if DVE.emit else None
        # CT hidden columns: psum staging -> CT
        DVE.wait(("pe", "hidT"))
        DVE.op(lambda: v.tensor_copy(CT.ap()[:, 0:HCH, :], ctcols8))
        DVE.mark("cth")
        # startup: copy even hidR stages out of psum (odd ones go to ACT)
        for k in range(0, 2 * BPC, 2):
            b, j = divmod(k, 2)
            DVE.wait(("pe", "hmm", b, j))
            DVE.op(lambda b=b, j=j: v.tensor_copy(
                hidR.ap()[:, b, j * 512:(j + 1) * 512], stage.ap()),
                drain=False)
            DVE.mark("hcp", b, j)
        for b in range(BPC):
            if b > 0:
                DVE.wait(("dma", "L", b))
            DVE.wait(("dve", "hcp", b, 0))
            DVE.wait(("act", "hcp", b, 1))
            for t in range(T):
                # prod slot reuse (8 slots): ACT must have consumed t-8
                NACT = 13
                if b == 0:
                    DVE.wait(("dma", "Lq", t // 4))
                if t >= 4 and t - 4 < NACT:
                    DVE.wait(("act", "acc", b, t - 4))
                elif b >= 1 and t < 4 and T - 4 + t < NACT:
                    DVE.wait(("act", "acc", b - 1, T - 4 + t))
                DVE.op(lambda b=b, t=t: v.tensor_mul(
                    prod[t % 4].ap(),
                    L[b % 2].ap()[:, t, :].bitcast(F32),
                    hidR.ap()[:, b, :]), drain=False)
                DVE.mark("mult", b, t)
                if t >= NACT:
                    if b >= 2:
                        DVE.wait(("act", "exp", b - 2))   # scores slot reuse
                    DVE.op(lambda b=b, t=t: v.reduce_sum(
                        scores[b % 2].ap()[:, t:t + 1], prod[t % 4].ap(),
                        axis=mybir.AxisListType.X))
                    DVE.mark("red", b, t)
                if t == 1 and b >= 1 and PHASE >= 3:
                    # 1/Z of the previous batch, early enough that ACT's
                    # cplo(b-1) (ahead of this batch's accs) can proceed
                    DVE.wait(("pe", "z", b - 1))
                    DVE.op(lambda b=b: v.reciprocal(rZ[(b - 1) % 2], Zps),
                           drain=False)
                    DVE.mark("recip", b - 1)
            DVE.mark("ttr", b)
            if PHASE < 2:
                continue
            if b >= 1:
                DVE.wait(("pe", "transp", b - 1))   # mp slot reuse
            DVE.wait(("act", "acc", b, 12))          # ACT's last score column
            DVE.op(lambda b=b: v.reduce_max(
                mp.ap(), scores[b % 2].ap(), axis=mybir.AxisListType.X))
            DVE.mark("rmax", b)
            DVE.wait(("pe", "transp", b))
            DVE.op(lambda b=b: v.reduce_max(
                negM1[b % 2], mpT, axis=mybir.AxisListType.X, negate=True))
            DVE.mark("rmax2", b)
        if PHASE >= 3:
            DVE.wait(("pe", "z", BPC - 1))
            DVE.op(lambda: v.reciprocal(rZ[(BPC - 1) % 2], Zps))
            DVE.mark("recip", BPC - 1)
        if PHASE < 6:
            return
        # final bias adds
        DVE.wait(("pe", "final"))
        DVE.op(lambda: v.tensor_add(
            out_sb[:, 0:512], acc_lo.ap()[0:BPC, :], bias[:, 0:512]))
        DVE.mark("bias_lo")
        DVE.wait(("dma", "bias"))
        DVE.op(lambda: v.tensor_add(
            out_sb[:, 512:1024], acc_hi.ap()[0:BPC, :], bias[:, 512:1024]))
        DVE.mark("bias_hi")

    def prog_act():
        a = ACT.eng if ACT.emit else None
        Copy = mybir.ActivationFunctionType.Copy
        Exp = mybir.ActivationFunctionType.Exp
        for k in range(1, 2 * BPC, 2):
            b, j = divmod(k, 2)
            ACT.wait(("pe", "hmm", b, j))
            ACT.op(lambda b=b, j=j: a.activation(
                out=hidR.ap()[:, b, j * 512:(j + 1) * 512], in_=stage2,
                func=Copy), drain=False)
            ACT.mark("hcp", b, j)
        for b in range(BPC):
            for t in range(13):
                ACT.wait(("dve", "mult", b, t))
                ACT.op(lambda b=b, t=t: a.activation(
                    out=dmy.ap()[:, t:t + 1].broadcast_to((128, H)),
                    in_=prod[t % 4].ap(),
                    func=Copy, accum_out=scores[b % 2].ap()[:, t:t + 1]),
                    drain=(t == 0))
                ACT.mark("acc", b, t)
            if b >= 2 and PHASE in (55, 6):
                ACT.wait(("pe", "attnT", b - 2))
                ACT.op(lambda b=b: a.activation(
                    out=CT.ap()[:, HCH:NCH, b - 2], in_=ctcols, func=Copy))
                ACT.mark("ctcp", b - 2)
            # previous batch's attn copies -- placed before negMcp/exp so
            # attnT(b-1) -> cphi(b-1) never chains through exp(b)
            if b >= 1 and PHASE >= 4:
                ACT.wait(("pe", "e2", b - 1))
                ACT.wait(("dve", "recip", b - 1))
                ACT.op(lambda b=b: a.activation(
                    out=attn_sb[(b - 1) % 2][0:1, 0:512], in_=acc_lo.ap()[0:1, :],
                    func=Copy, scale=rZ[(b - 1) % 2]))
                ACT.mark("cplo", b - 1)
                ACT.op(lambda b=b: a.activation(
                    out=attn_sb[(b - 1) % 2][0:1, 512:1024], in_=acc_hi.ap()[0:1, :],
                    func=Copy, scale=rZ[(b - 1) % 2]))
                ACT.mark("cphi", b - 1)

            if PHASE >= 2:
                ACT.wait(("pe", "bcast", b))
                ACT.op(lambda b=b: a.activation(
                    out=negM[b % 2].ap(), in_=negM_bc, func=Copy))
                ACT.mark("negMcp", b)
            if PHASE >= 3:
                if b >= 2 and PHASE >= 4:
                    ACT.wait(("pe", "e2", b - 2))    # wexp/zp slot reuse
                ACT.op(lambda b=b: a.activation(
                    out=wexp[b % 2].ap(), in_=scores[b % 2].ap(), func=Exp,
                    bias=negM[b % 2].ap(), scale=1.0, accum_out=zp[b % 2].ap()))
                ACT.mark("exp", b)
            if PHASE < 4:
                continue
        for b in (BPC - 1,):
            ACT.wait(("pe", "e2", b))
            ACT.wait(("dve", "recip", b))
            ACT.op(lambda b=b: a.activation(
                out=attn_sb[b % 2][0:1, 0:512], in_=acc_lo.ap()[0:1, :],
                func=Copy, scale=rZ[b % 2]))
            ACT.mark("cplo", b)
            ACT.op(lambda b=b: a.activation(
                out=attn_sb[b % 2][0:1, 512:1024], in_=acc_hi.ap()[0:1, :],
                func=Copy, scale=rZ[b % 2]))
            ACT.mark("cphi", b)
        if PHASE in (55, 6):
            for b in (BPC - 2, BPC - 1):
                ACT.wait(("pe", "attnT", b))
                ACT.op(lambda b=b: a.activation(
                    out=CT.ap()[:, HCH:NCH, b], in_=ctcols, func=Copy))
                ACT.mark("ctcp", b)

    progs = [
        (GPS, prog_gps), (DMA, prog_dma), (PE, prog_pe),
        (DVE, prog_dve), (ACT, prog_act),
    ]

    # pass 1: count
    for pr, fn in progs:
        pr.begin(emit=False)
        fn()

    # pass 2: emit
    counts.clear()
    sem_names = ["pe", "dve", "act", "gps", "hid", "bias", "l0", "l1", "wt",
                 "outd", "gdma", "q0", "q1", "q2", "q3"]
    with nc.Block() as block:
        for sn in sem_names:
            sems[sn] = nc.alloc_semaphore(name=f"{sn}_sem")

        @block.gpsimd
        def _(eng):
            GPS.begin(eng=eng, emit=True)
            prog_gps()

        @block.sync
        def _(eng):
            DMA.begin(eng=eng, emit=True)
            prog_dma()

        @block.tensor
        def _(eng):
            PE.begin(eng=eng, emit=True)
            prog_pe()

        @block.vector
        def _(eng):
            DVE.begin(eng=eng, emit=True)
            prog_dve()

        @block.scalar
        def _(eng):
            ACT.begin(eng=eng, emit=True)
            prog_act()

    return nc


def kernel(lstm_output, hidden, W_combine, b_combine):
    global _cached_nc, last_results
    lstm_output = np.asarray(lstm_output, dtype=np.float32)
    hidden = np.asarray(hidden, dtype=np.float32)
    W_combine = np.asarray(W_combine, dtype=np.float32)
    b_combine = np.asarray(b_combine, dtype=np.float32)

    if _cached_nc is None:
        _cached_nc = _build_program()
    nc = _cached_nc

    wt_host = np.ascontiguousarray(W_combine.T)
    in_maps = []
    for i in range(NCORES):
        sl = slice(i * BPC, (i + 1) * BPC)
        in_maps.append({
            "lstm_output": np.ascontiguousarray(lstm_output[sl]),
            "hidden": np.ascontiguousarray(hidden[sl]),
            "w_t": wt_host,
            "b_combine": b_combine,
        })
    res = run_bass_kernel_spmd(nc, in_maps, core_ids=list(range(NCORES)))
    last_results = res
    return np.concatenate([res.results[i]["out"] for i in range(NCORES)], axis=0)



# revision 8
# speedup vs baseline: 1.4719x; 1.4719x over previous
"""Trainium2 Bass kernel for nn_Attention (dense_transformer, ridge regime).

Computation per batch b:
    scores[s]  = <lstm_output[b,s,:], hidden[b,:]>          # [S]
    w          = softmax(scores)                            # [S]
    attn[h]    = sum_s w[s] * lstm_output[b,s,h]            # [H]
    out[b]     = [hidden[b], attn] @ W_combine.T + b_combine

Sharding: data-parallel over batch B=64 across 8 cores (8 batches/core).

v2 design (fp16 data path; measured numpy rel-err ~1.1e-3 vs 2e-2 gate):
  - Host converts lstm_output to fp16 and relays out to partition-major
    [BPC, 128, T, H] so each batch is one contiguous-per-partition 4MB DMA
    (issued in 1MB quarters for pipelining; L is triple-buffered).
    HBM traffic per core: 32MB lstm + 4MB W.T(fp16) + small = ~36MB.
  - Scores: 16 t-columns split across engines per batch:
      t 0..8  -> DVE tensor_mul (fp16 2x mode) + ACT accum-copy
      t 9..13 -> GPSIMD fused scalar_tensor_tensor (mult+mult, accum sum)
      t 14,15 -> DVE fused tensor_tensor_reduce
    All accumulate f32 scores [128, T] directly (no separate reduce pass).
  - Softmax max chain, exp (fp16 wexp out + f32 Z accum), einsum2 and the
    final projection as fp16 PE matmuls; transposes stay f32 into PSUM
    with dtype conversion on the ACT/DVE copy-out (no fp16 PSUM anywhere).
  - W.T fp16 [128, 16, 1024] loaded mid-stream into its own SBUF buffer
    (no L-slot reuse, no tail stall waiting on an 8MB f32 W load).
"""

import numpy as np

import concourse.bass as bass
from concourse import bass_isa, library_config, mybir
from concourse.bass_utils import run_bass_kernel_spmd

F32 = mybir.dt.float32
F16 = mybir.dt.float16

B, S, H = 64, 2048, 1024
NCORES = 8
BPC = B // NCORES          # batches per core
T = S // 128               # s-tiles per batch
NCH = (2 * H) // 128       # 16 chunks of the combined dim
HCH = H // 128             # 8 chunks of one H
NQ = 4                     # DMA quarters per batch
TQ = T // NQ               # t-tiles per quarter
NL = 3                     # L buffer slots (triple buffer)

ACT_T = list(range(0, 12))     # DVE mult + ACT accum columns
GPS_T = []                     # GPSIMD columns (walrus rejects stt on Pool)
TTR_T = [12, 13, 14, 15]       # DVE mult + DVE reduce columns

_cached_nc = None
last_results = None


def _build_program():
    nc = bass.Bass()

    lstm_d = nc.declare_dram_parameter("lstm16", [BPC, 128, T, H], F16, isOutput=False)
    hid_d = nc.declare_dram_parameter("hidden", [BPC, H], F32, isOutput=False)
    hid16_d = nc.declare_dram_parameter("hidden16", [BPC, H], F16, isOutput=False)
    wt_d = nc.declare_dram_parameter("w_t16", [128, NCH, H], F16, isOutput=False)
    b_d = nc.declare_dram_parameter("b_combine", [H], F32, isOutput=False)
    out_d = nc.declare_dram_parameter("out", [BPC, H], F32, isOutput=True)

    # ---- SBUF ----
    L = [nc.alloc_sbuf_tensor(f"L{i}", [128, T, H], F16) for i in range(NL)]  # 3x4MB
    WT = nc.alloc_sbuf_tensor("WT", [128, NCH, H], F16)                       # 4MB
    hid_t = nc.alloc_sbuf_tensor("hid", [BPC, H], F32)
    hid = hid_t.ap()
    hid16_t = nc.alloc_sbuf_tensor("hid16", [BPC, H], F16)
    hid16 = hid16_t.ap()
    bias_t = nc.alloc_sbuf_tensor("bias", [BPC, H], F32)
    bias = bias_t.ap()
    out_t = nc.alloc_sbuf_tensor("out_sb", [BPC, H], F32)
    out_sb = out_t.ap()
    hidR = nc.alloc_sbuf_tensor("hidR", [128, BPC, H], F16)   # 2MB bcast hidden
    prod = [nc.alloc_sbuf_tensor(f"prod{i}", [128, H], F16) for i in range(4)]
    dprod = [nc.alloc_sbuf_tensor(f"dprod{i}", [128, H], F16) for i in range(4)]
    dmy = nc.alloc_sbuf_tensor("dmy", [128, T], F32)
    CT = nc.alloc_sbuf_tensor("CT", [128, NCH, BPC], F16)     # combined^T
    scores = [nc.alloc_sbuf_tensor(f"scores{i}", [128, T], F32) for i in range(2)]
    wexp = [nc.alloc_sbuf_tensor(f"wexp{i}", [128, T], F16) for i in range(2)]
    zp = [nc.alloc_sbuf_tensor(f"zp{i}", [128, 1], F32) for i in range(2)]
    mp = nc.alloc_sbuf_tensor("mp", [128, 1], F32)
    negM1_t = nc.alloc_sbuf_tensor("negM1s", [1, 2], F32)
    negM1 = [negM1_t.ap()[0:1, i:i + 1] for i in range(2)]
    negM = [nc.alloc_sbuf_tensor(f"negM{i}", [128, 1], F32) for i in range(2)]
    rZ_t = nc.alloc_sbuf_tensor("rZs", [1, 2], F32)
    rZ = [rZ_t.ap()[0:1, i:i + 1] for i in range(2)]
    ones128 = nc.alloc_sbuf_tensor("ones128", [128, 1], F32)
    attn2 = nc.alloc_sbuf_tensor("attn2", [1, 2 * H], F32)
    attn_sb = [attn2.ap()[0:1, i * H:(i + 1) * H] for i in range(2)]
    ones_col = nc.alloc_sbuf_tensor("ones_col", [1, 128], F32)
    ident = nc.alloc_sbuf_tensor("ident", [128, 128], F32)
    sel16 = nc.alloc_sbuf_tensor("sel16", [BPC, BPC, 128], F16)  # sel[k,b,:]=(k==b)

    # ---- PSUM: one bank per concurrent PE write target ----
    acc_lo = nc.alloc_psum_tensor("acc_lo", [BPC, 512], F32)  # einsum2 row 0 / final
    acc_hi = nc.alloc_psum_tensor("acc_hi", [BPC, 512], F32)
    ct8_t = nc.alloc_psum_tensor("ct8", [128, HCH, BPC], F32) # hidT staging
    ctc_t = nc.alloc_psum_tensor("ctc", [128, 512], F32)      # attnT transposes
    stage = nc.alloc_psum_tensor("stage", [128, 512], F32)    # hidR staging mms
    mpT_t = nc.alloc_psum_tensor("mpT", [1, 128], F32)        # transp target
    negM_t = nc.alloc_psum_tensor("negMbc", [128, 1], F32)    # bcast mm target
    Zps_t = nc.alloc_psum_tensor("Zps", [1, 1], F32)          # Z mm target
    mpT = mpT_t.ap()
    negM_bc = negM_t.ap()
    Zps = Zps_t.ap()
    ctcols8 = ct8_t.ap()
    ctcols = ctc_t.ap()[:, 0:HCH]
    stage2 = ctc_t.ap()   # startup-only reuse of the attnT bank

    # ---------------- two-pass emission ----------------
    ev = {}
    sems = {}
    counts = {}

    class Prog:
        def __init__(self, name):
            self.name = name
            self.emit = False
            self.eng = None
            self.hwm = {}
            self.auto_drain = name in ("dve", "act", "gps")
            self.first_op = True

        def begin(self, eng=None, emit=False):
            self.emit = emit
            self.eng = eng
            self.hwm = {}
            self.first_op = True

        def wait(self, key):
            """key: event tuple, or (sem_name, value) pair."""
            if len(key) == 2 and isinstance(key[1], int) and key[0] in (
                    "pe", "dve", "act", "gps", "hid", "hid16", "bias",
                    "l0", "l1", "l2", "wt", "outd"):
                sname, val = key
            else:
                if self.emit and key not in ev:
                    raise KeyError(f"wait on unknown event {key}")
                sname, val = ev.get(key, (None, 0))
            if val <= 0 or sname is None:
                return
            if self.hwm.get(sname, -1) >= val:
                return
            self.hwm[sname] = val
            if self.emit:
                self.eng.wait_ge(sems[sname], val)

        def op(self, fn, inc=1, sem=None, drain=None):
            sname = sem or self.name
            counts[sname] = counts.get(sname, 0) + inc
            if self.emit:
                do_drain = self.auto_drain if drain is None else drain
                if do_drain and not self.first_op:
                    self.eng.drain()
                inst = fn()
                inst.then_inc(sems[sname], inc)
            self.first_op = False

        def mark(self, *key, sem=None):
            sname = sem or self.name
            ev[(self.name,) + tuple(key)] = (sname, counts.get(sname, 0))

    DMA, PE, DVE, ACT, GPS = Prog("dma"), Prog("pe"), Prog("dve"), Prog("act"), Prog("gps")

    bias_src = b_d[:]
    bias_bcast = bass.AP(
        tensor=bias_src.tensor,
        offset=bias_src.offset,
        ap=[[0, BPC]] + list(bias_src.ap),
    )

    def prog_gps():
        g = GPS.eng if GPS.emit else None
        GPS.op(lambda: g.memset(ones_col.ap(), 1.0))
        GPS.op(lambda: g.memset(ones128.ap(), 1.0))
        GPS.op(lambda: g.memset(ident.ap(), 0.0))
        GPS.op(lambda: g.affine_select(
            out=ident.ap(), in_=ident.ap(),
            compare_op=mybir.AluOpType.not_equal, fill=1.0, base=0,
            pattern=[[-1, 128]], channel_multiplier=1))
        GPS.op(lambda: g.memset(sel16.ap(), 0.0), drain=True)
        GPS.op(lambda: g.affine_select(
            out=sel16.ap(), in_=sel16.ap(),
            compare_op=mybir.AluOpType.not_equal, fill=1.0, base=0,
            pattern=[[-1, BPC], [0, 128]], channel_multiplier=1), drain=True)
        GPS.mark("setup")

    def prog_dma():
        d = DMA.eng if DMA.emit else None
        DMA.op(lambda: d.dma_start(out=hid, in_=hid_d[:]), inc=16, sem="hid")
        DMA.mark("hid", sem="hid")
        DMA.op(lambda: d.dma_start(out=hid16, in_=hid16_d[:]), inc=16, sem="hid16")
        DMA.mark("hid16", sem="hid16")
        DMA.op(lambda: d.dma_start(out=bias, in_=bias_bcast), inc=16, sem="bias")
        DMA.mark("bias", sem="bias")
        for b in range(BPC):
            if b >= NL:
                DMA.wait(("pe", "e2", b - NL))
            src = lstm_d[b]
            for q in range(NQ):
                DMA.op(lambda src=src, b=b, q=q: d.dma_start(
                    out=L[b % NL].ap()[:, TQ * q:TQ * (q + 1), :],
                    in_=src[:, TQ * q:TQ * (q + 1), :]),
                    inc=16, sem=f"l{b % NL}")
                DMA.mark("Lq", b, q, sem=f"l{b % NL}")
            DMA.mark("L", b, sem=f"l{b % NL}")
            if b == 2:
                DMA.op(lambda: d.dma_start(out=WT.ap(), in_=wt_d[:]),
                       inc=16, sem="wt")
                DMA.mark("wt", sem="wt")
        DMA.wait(("dve", "bias_hi"))
        DMA.op(lambda: d.dma_start(out=out_d[:], in_=out_sb), inc=16, sem="outd")
        DMA.wait(("outd", counts.get("outd", 0)))

    def prog_pe():
        p = PE.eng if PE.emit else None
        PE.wait(("gps", "setup"))
        PE.wait(("dma", "hid"))
        # hidden^T -> CT chunks 0..7 staging (psum)
        for c in range(HCH):
            PE.op(lambda c=c: p.transpose(
                ctcols8[:, c, :], hid[0:BPC, c * 128:(c + 1) * 128],
                ident.ap()[0:BPC, 0:BPC]))
        PE.mark("hidT")
        # replicate hidden rows across partitions: sel-matmul (fp16) into
        # psum staging banks; DVE/ACT copy out to hidR fp16
        PE.wait(("dma", "hid16"))
        for k in range(2 * BPC):
            b, j = divmod(k, 2)
            if k == 1:
                PE.wait(("dve", "cth"))   # ctc bank free of setup readers
            if k > 1:
                pb, pj = divmod(k - 2, 2)
                PE.wait(("dve" if k % 2 == 0 else "act", "hcp", pb, pj))
            tgt = stage.ap() if k % 2 == 0 else stage2
            PE.op(lambda b=b, j=j, tgt=tgt: p.matmul(
                tgt, lhsT=sel16.ap()[:, b, :],
                rhs=hid16[0:BPC, j * 512:(j + 1) * 512],
                start=True, stop=True))
            PE.mark("hmm", b, j)
        PE.wait(("dve", "rmax", 0))
        PE.op(lambda: p.transpose(mpT, mp.ap(), ident.ap()))
        PE.mark("transp", 0)
        for b in range(BPC):
            PE.wait(("dve", "rmax2", b))
            PE.op(lambda b=b: p.matmul(
                negM_bc, lhsT=ones_col.ap(), rhs=negM1[b % 2],
                start=True, stop=True))
            PE.mark("bcast", b)
            if b >= 1:
                PE.wait(("dve", "recip", b - 1))   # Zps slot reuse
            PE.wait(("act", "exp", b))
            PE.op(lambda b=b: p.matmul(
                Zps, lhsT=zp[b % 2].ap(), rhs=ones128.ap(),
                start=True, stop=True))
            PE.mark("z", b)
            # einsum2: attn_unnorm = sum_s w[s] * L[s, :]
            if b >= 1:
                PE.wait(("act", "cphi", b - 1))    # acc bank reuse
            for t in range(T):
                PE.op(lambda b=b, t=t: p.matmul(
                    acc_lo.ap()[0:1, :],
                    lhsT=wexp[b % 2].ap()[:, t:t + 1],
                    rhs=L[b % NL].ap()[:, t, 0:512],
                    start=(t == 0), stop=(t == T - 1)))
                PE.op(lambda b=b, t=t: p.matmul(
                    acc_hi.ap()[0:1, :],
                    lhsT=wexp[b % 2].ap()[:, t:t + 1],
                    rhs=L[b % NL].ap()[:, t, 512:1024],
                    start=(t == 0), stop=(t == T - 1)))
            PE.mark("e2", b)
            if b + 1 < BPC:
                PE.wait(("dve", "rmax", b + 1))
                PE.op(lambda: p.transpose(mpT, mp.ap(), ident.ap()))
                PE.mark("transp", b + 1)
            # attn row -> columns (chunk transposes via K=1 matmuls)
            PE.wait(("act", "cphi", b))
            if b >= 1:
                PE.wait(("act", "ctcp", b - 1))
            for c in range(HCH):
                PE.op(lambda b=b, c=c: p.transpose(
                    ctcols[:, c:c + 1],
                    attn_sb[b % 2][0:1, c * 128:(c + 1) * 128],
                    ones_col.ap()[0:1, 0:1]))
            PE.mark("attnT", b)
        # final projection (fp16): combined^T @ W^T chunks
        PE.wait(("act", "ctcp", BPC - 1))
        PE.wait(("dma", "wt"))
        for c in range(NCH):
            PE.op(lambda c=c: p.matmul(
                acc_lo.ap()[0:BPC, :],
                lhsT=CT.ap()[:, c, :],
                rhs=WT.ap()[:, c, 0:512],
                start=(c == 0), stop=(c == NCH - 1)))
            PE.op(lambda c=c: p.matmul(
                acc_hi.ap()[0:BPC, :],
                lhsT=CT.ap()[:, c, :],
                rhs=WT.ap()[:, c, 512:1024],
                start=(c == 0), stop=(c == NCH - 1)))
        PE.mark("final")

    def prog_dve():
        v = DVE.eng if DVE.emit else None
        # CT hidden columns: psum staging -> CT (f32 -> fp16 convert)
        DVE.wait(("pe", "hidT"))
        DVE.op(lambda: v.tensor_copy(CT.ap()[:, 0:HCH, :], ctcols8))
        DVE.mark("cth")
        # startup: copy even hidR stages out of psum (odd ones go to ACT)
        for k in range(0, 2 * BPC, 2):
            b, j = divmod(k, 2)
            DVE.wait(("pe", "hmm", b, j))
            DVE.op(lambda b=b, j=j: v.tensor_copy(
                hidR.ap()[:, b, j * 512:(j + 1) * 512], stage.ap()),
                drain=False)
            DVE.mark("hcp", b, j)
        for b in range(BPC):
            DVE.wait(("dve", "hcp", b, 0))
            DVE.wait(("act", "hcp", b, 1))
            for t in ACT_T:
                DVE.wait(("dma", "Lq", b, t // TQ))
                # prod slot reuse (4 slots): ACT must have consumed t-4
                if t >= 4:
                    DVE.wait(("act", "acc", b, t - 4))
                elif b >= 1:
                    DVE.wait(("act", "acc", b - 1,
                              ACT_T[-1] if t == 0 else t + 4))
                DVE.op(lambda b=b, t=t: v.tensor_mul(
                    prod[t % 4].ap(),
                    L[b % NL].ap()[:, t, :],
                    hidR.ap()[:, b, :]), drain=False)
                DVE.mark("mult", b, t)
            if b >= 2:
                DVE.wait(("act", "exp", b - 2))   # scores slot reuse
            for t in TTR_T:
                DVE.wait(("dma", "Lq", b, t // TQ))
                DVE.op(lambda b=b, t=t: v.tensor_mul(
                    dprod[t % 4].ap(),
                    L[b % NL].ap()[:, t, :],
                    hidR.ap()[:, b, :]), drain=False)
            for t in TTR_T:
                DVE.op(lambda b=b, t=t: v.reduce_sum(
                    scores[b % 2].ap()[:, t:t + 1], dprod[t % 4].ap(),
                    axis=mybir.AxisListType.X))
                DVE.mark("ttr", b, t)
            if b >= 1:
                # 1/Z of the previous batch, before rmax so ACT's cplo(b-1)
                # can proceed promptly
                DVE.wait(("pe", "z", b - 1))
                DVE.op(lambda b=b: v.reciprocal(rZ[(b - 1) % 2], Zps),
                       drain=False)
                DVE.mark("recip", b - 1)
            if b >= 1:
                DVE.wait(("pe", "transp", b - 1))   # mp slot reuse
            DVE.wait(("act", "acc", b, ACT_T[-1]))
            DVE.op(lambda b=b: v.reduce_max(
                mp.ap(), scores[b % 2].ap(), axis=mybir.AxisListType.X))
            DVE.mark("rmax", b)
            DVE.wait(("pe", "transp", b))
            DVE.op(lambda b=b: v.reduce_max(
                negM1[b % 2], mpT, axis=mybir.AxisListType.X, negate=True))
            DVE.mark("rmax2", b)
        DVE.wait(("pe", "z", BPC - 1))
        DVE.op(lambda: v.reciprocal(rZ[(BPC - 1) % 2], Zps))
        DVE.mark("recip", BPC - 1)
        # final bias adds
        DVE.wait(("pe", "final"))
        DVE.op(lambda: v.tensor_add(
            out_sb[:, 0:512], acc_lo.ap()[0:BPC, :], bias[:, 0:512]))
        DVE.mark("bias_lo")
        DVE.wait(("dma", "bias"))
        DVE.op(lambda: v.tensor_add(
            out_sb[:, 512:1024], acc_hi.ap()[0:BPC, :], bias[:, 512:1024]))
        DVE.mark("bias_hi")

    def prog_act():
        a = ACT.eng if ACT.emit else None
        Copy = mybir.ActivationFunctionType.Copy
        Exp = mybir.ActivationFunctionType.Exp
        for k in range(1, 2 * BPC, 2):
            b, j = divmod(k, 2)
            ACT.wait(("pe", "hmm", b, j))
            ACT.op(lambda b=b, j=j: a.activation(
                out=hidR.ap()[:, b, j * 512:(j + 1) * 512], in_=stage2,
                func=Copy), drain=False)
            ACT.mark("hcp", b, j)
        for b in range(BPC):
            for t in ACT_T:
                ACT.wait(("dve", "mult", b, t))
                ACT.op(lambda b=b, t=t: a.activation(
                    out=dmy.ap()[:, t:t + 1].broadcast_to((128, H)),
                    in_=prod[t % 4].ap(),
                    func=Copy, accum_out=scores[b % 2].ap()[:, t:t + 1]),
                    drain=(t == 0))
                ACT.mark("acc", b, t)
            if b >= 2:
                ACT.wait(("pe", "attnT", b - 2))
                ACT.op(lambda b=b: a.activation(
                    out=CT.ap()[:, HCH:NCH, b - 2], in_=ctcols, func=Copy))
                ACT.mark("ctcp", b - 2)
            # previous batch's attn copies -- placed before negMcp/exp so
            # attnT(b-1) -> cphi(b-1) never chains through exp(b)
            if b >= 1:
                ACT.wait(("pe", "e2", b - 1))
                ACT.wait(("dve", "recip", b - 1))
                ACT.op(lambda b=b: a.activation(
                    out=attn_sb[(b - 1) % 2][0:1, 0:512], in_=acc_lo.ap()[0:1, :],
                    func=Copy, scale=rZ[(b - 1) % 2]))
                ACT.mark("cplo", b - 1)
                ACT.op(lambda b=b: a.activation(
                    out=attn_sb[(b - 1) % 2][0:1, 512:1024], in_=acc_hi.ap()[0:1, :],
                    func=Copy, scale=rZ[(b - 1) % 2]))
                ACT.mark("cphi", b - 1)
            ACT.wait(("pe", "bcast", b))
            ACT.op(lambda b=b: a.activation(
                out=negM[b % 2].ap(), in_=negM_bc, func=Copy))
            ACT.mark("negMcp", b)
            if b >= 2:
                ACT.wait(("pe", "e2", b - 2))    # wexp/zp slot reuse
            ACT.op(lambda b=b: a.activation(
                out=wexp[b % 2].ap(), in_=scores[b % 2].ap(), func=Exp,
                bias=negM[b % 2].ap(), scale=1.0, accum_out=zp[b % 2].ap()))
            ACT.mark("exp", b)
        for b in (BPC - 1,):
            ACT.wait(("pe", "e2", b))
            ACT.wait(("dve", "recip", b))
            ACT.op(lambda b=b: a.activation(
                out=attn_sb[b % 2][0:1, 0:512], in_=acc_lo.ap()[0:1, :],
                func=Copy, scale=rZ[b % 2]))
            ACT.mark("cplo", b)
            ACT.op(lambda b=b: a.activation(
                out=attn_sb[b % 2][0:1, 512:1024], in_=acc_hi.ap()[0:1, :],
                func=Copy, scale=rZ[b % 2]))
            ACT.mark("cphi", b)
        for b in (BPC - 2, BPC - 1):
            ACT.wait(("pe", "attnT", b))
            ACT.op(lambda b=b: a.activation(
                out=CT.ap()[:, HCH:NCH, b], in_=ctcols, func=Copy))
            ACT.mark("ctcp", b)

    progs = [
        (GPS, prog_gps), (DMA, prog_dma), (PE, prog_pe),
        (DVE, prog_dve), (ACT, prog_act),
    ]

    # pass 1: count
    for pr, fn in progs:
        pr.begin(emit=False)
        fn()

    # pass 2: emit
    counts.clear()
    sem_names = ["pe", "dve", "act", "gps", "hid", "hid16", "bias",
                 "l0", "l1", "l2", "wt", "outd"]
    with nc.Block() as block:
        for sn in sem_names:
            sems[sn] = nc.alloc_semaphore(name=f"{sn}_sem")

        @block.gpsimd
        def _(eng):
            GPS.begin(eng=eng, emit=True)
            prog_gps()

        @block.sync
        def _(eng):
            DMA.begin(eng=eng, emit=True)
            prog_dma()

        @block.tensor
        def _(eng):
            PE.begin(eng=eng, emit=True)
            prog_pe()

        @block.vector
        def _(eng):
            DVE.begin(eng=eng, emit=True)
            prog_dve()

        @block.scalar
        def _(eng):
            ACT.begin(eng=eng, emit=True)
            prog_act()

    return nc


def kernel(lstm_output, hidden, W_combine, b_combine):
    global _cached_nc, last_results
    lstm_output = np.asarray(lstm_output, dtype=np.float32)
    hidden = np.asarray(hidden, dtype=np.float32)
    W_combine = np.asarray(W_combine, dtype=np.float32)
    b_combine = np.asarray(b_combine, dtype=np.float32)

    if _cached_nc is None:
        _cached_nc = _build_program()
    nc = _cached_nc

    # fp16 partition-major relayout: [B, S, H] -> [B, 128, T, H]
    l16 = lstm_output.astype(np.float16).reshape(B, T, 128, H).transpose(0, 2, 1, 3)
    wt16 = np.ascontiguousarray(
        W_combine.T.astype(np.float16).reshape(NCH, 128, H).transpose(1, 0, 2))
    hid16 = hidden.astype(np.float16)

    in_maps = []
    for i in range(NCORES):
        sl = slice(i * BPC, (i + 1) * BPC)
        in_maps.append({
            "lstm16": np.ascontiguousarray(l16[sl]),
            "hidden": np.ascontiguousarray(hidden[sl]),
            "hidden16": np.ascontiguousarray(hid16[sl]),
            "w_t16": wt16,
            "b_combine": b_combine,
        })
    res = run_bass_kernel_spmd(nc, in_maps, core_ids=list(range(NCORES)))
    last_results = res
    return np.concatenate([res.results[i]["out"] for i in range(NCORES)], axis=0)


# revision 10
# speedup vs baseline: 1.7999x; 1.2228x over previous
"""Trainium2 Bass kernel for nn_Attention (dense_transformer, ridge regime).

Computation per batch b:
    scores[s]  = <lstm_output[b,s,:], hidden[b,:]>          # [S]
    w          = softmax(scores)                            # [S]
    attn[h]    = sum_s w[s] * lstm_output[b,s,h]            # [H]
    out[b]     = [hidden[b], attn] @ W_combine.T + b_combine

Sharding: data-parallel over batch B=64 across 8 cores (8 batches/core).

v3 design (fp16 data path; measured rel-err ~1.2e-3 vs 2e-2 gate):
  - Host converts lstm_output to fp16, partition-major [BPC, 128, T, H]:
    each batch one contiguous-per-partition 4MB DMA in 1MB quarters,
    L triple-buffered. HBM/core: 32MB lstm + 4MB W.T + 2MB hidR + small.
  - hidR (hidden replicated across partitions) comes from one broadcast
    DMA (stride-0 partition read of hidden16) -- no sel-matmul staging.
  - Scores per batch: 4 quad-multiplies on DVE (fp16 2x, one per DMA
    quarter, in1 = hidR stride-0 broadcast over the t dim), each into its
    own prodQ buffer so ACT never paces DVE. Columns 0-10 reduced by ACT
    accum-copies; 11-15 by DVE (single + quad tensor_reduce).
  - Softmax max chain via PE transposes; exp on ACT (fp16 wexp out, f32 Z
    accum); einsum2 and final projection as fp16 PE matmuls; transposes
    stay f32 into PSUM, converting on the ACT/DVE copy-out.
  - W.T fp16 [128, 16, 1024] loads mid-stream into its own SBUF buffer.
"""

import numpy as np

import concourse.bass as bass
from concourse import bass_isa, library_config, mybir
from concourse.bass_utils import run_bass_kernel_spmd

F32 = mybir.dt.float32
F16 = mybir.dt.float16

B, S, H = 64, 2048, 1024
NCORES = 8
BPC = B // NCORES          # batches per core
T = S // 128               # s-tiles per batch
NCH = (2 * H) // 128       # 16 chunks of the combined dim
HCH = H // 128             # 8 chunks of one H
NQ = 4                     # DMA quarters / mult quads per batch
TQ = T // NQ               # t-tiles per quarter
NL = 3                     # L buffer slots (triple buffer)

NACT = 11                  # score cols 0..NACT-1 on ACT; rest on DVE

_cached_nc = None
last_results = None


def _build_program():
    nc = bass.Bass()

    lstm_d = nc.declare_dram_parameter("lstm16", [BPC, 128, T, H], F16, isOutput=False)
    hid_d = nc.declare_dram_parameter("hidden", [BPC, H], F32, isOutput=False)
    hid16_d = nc.declare_dram_parameter("hidden16", [BPC, H], F16, isOutput=False)
    wt_d = nc.declare_dram_parameter("w_t16", [128, NCH, H], F16, isOutput=False)
    b_d = nc.declare_dram_parameter("b_combine", [H], F32, isOutput=False)
    out_d = nc.declare_dram_parameter("out", [BPC, H], F32, isOutput=True)

    # ---- SBUF ----
    L = [nc.alloc_sbuf_tensor(f"L{i}", [128, T, H], F16) for i in range(NL)]  # 3x4MB
    WT = nc.alloc_sbuf_tensor("WT", [128, NCH, H], F16)                       # 4MB
    hid_t = nc.alloc_sbuf_tensor("hid", [BPC, H], F32)
    hid = hid_t.ap()
    bias_t = nc.alloc_sbuf_tensor("bias", [BPC, H], F32)
    bias = bias_t.ap()
    out_t = nc.alloc_sbuf_tensor("out_sb", [BPC, H], F32)
    out_sb = out_t.ap()
    hidR = nc.alloc_sbuf_tensor("hidR", [128, BPC, H], F16)   # 2MB bcast hidden
    prodQ = [nc.alloc_sbuf_tensor(f"prodQ{i}", [128, TQ, H], F16) for i in range(NQ)]
    dmy = nc.alloc_sbuf_tensor("dmy", [128, T], F32)
    CT = nc.alloc_sbuf_tensor("CT", [128, NCH, BPC], F16)     # combined^T
    scores = [nc.alloc_sbuf_tensor(f"scores{i}", [128, T], F32) for i in range(2)]
    wexp = [nc.alloc_sbuf_tensor(f"wexp{i}", [128, T], F16) for i in range(2)]
    zp = [nc.alloc_sbuf_tensor(f"zp{i}", [128, 1], F32) for i in range(2)]
    mp = nc.alloc_sbuf_tensor("mp", [128, 1], F32)
    negM1_t = nc.alloc_sbuf_tensor("negM1s", [1, 2], F32)
    negM1 = [negM1_t.ap()[0:1, i:i + 1] for i in range(2)]
    negM = [nc.alloc_sbuf_tensor(f"negM{i}", [128, 1], F32) for i in range(2)]
    rZ_t = nc.alloc_sbuf_tensor("rZs", [1, 2], F32)
    rZ = [rZ_t.ap()[0:1, i:i + 1] for i in range(2)]
    ones128 = nc.alloc_sbuf_tensor("ones128", [128, 1], F32)
    attn2 = nc.alloc_sbuf_tensor("attn2", [1, 2 * H], F32)
    attn_sb = [attn2.ap()[0:1, i * H:(i + 1) * H] for i in range(2)]
    ones_col = nc.alloc_sbuf_tensor("ones_col", [1, 128], F32)
    ident = nc.alloc_sbuf_tensor("ident", [128, 128], F32)

    # ---- PSUM ----
    acc_lo = nc.alloc_psum_tensor("acc_lo", [BPC, 512], F32)  # einsum2 row 0 / final
    acc_hi = nc.alloc_psum_tensor("acc_hi", [BPC, 512], F32)
    ct8_t = nc.alloc_psum_tensor("ct8", [128, HCH, BPC], F32) # hidT staging
    ctc_t = nc.alloc_psum_tensor("ctc", [128, 512], F32)      # attnT transposes
    mpT_t = nc.alloc_psum_tensor("mpT", [1, 128], F32)        # transp target
    negM_t = nc.alloc_psum_tensor("negMbc", [128, 1], F32)    # bcast mm target
    Zps_t = nc.alloc_psum_tensor("Zps", [1, 1], F32)          # Z mm target
    mpT = mpT_t.ap()
    negM_bc = negM_t.ap()
    Zps = Zps_t.ap()
    ctcols8 = ct8_t.ap()
    ctcols = ctc_t.ap()[:, 0:HCH]

    # ---------------- two-pass emission ----------------
    ev = {}
    sems = {}
    counts = {}

    class Prog:
        def __init__(self, name):
            self.name = name
            self.emit = False
            self.eng = None
            self.hwm = {}
            self.auto_drain = name in ("dve", "act", "gps")
            self.first_op = True

        def begin(self, eng=None, emit=False):
            self.emit = emit
            self.eng = eng
            self.hwm = {}
            self.first_op = True

        def wait(self, key):
            """key: event tuple, or (sem_name, value) pair."""
            if len(key) == 2 and isinstance(key[1], int) and key[0] in (
                    "pe", "dve", "act", "gps", "hid", "hbc", "bias",
                    "l0", "l1", "l2", "wt", "outd"):
                sname, val = key
            else:
                if self.emit and key not in ev:
                    raise KeyError(f"wait on unknown event {key}")
                sname, val = ev.get(key, (None, 0))
            if val <= 0 or sname is None:
                return
            if self.hwm.get(sname, -1) >= val:
                return
            self.hwm[sname] = val
            if self.emit:
                self.eng.wait_ge(sems[sname], val)

        def op(self, fn, inc=1, sem=None, drain=None):
            sname = sem or self.name
            counts[sname] = counts.get(sname, 0) + inc
            if self.emit:
                do_drain = self.auto_drain if drain is None else drain
                if do_drain and not self.first_op:
                    self.eng.drain()
                inst = fn()
                inst.then_inc(sems[sname], inc)
            self.first_op = False

        def mark(self, *key, sem=None):
            sname = sem or self.name
            ev[(self.name,) + tuple(key)] = (sname, counts.get(sname, 0))

    DMA, PE, DVE, ACT, GPS = Prog("dma"), Prog("pe"), Prog("dve"), Prog("act"), Prog("gps")

    bias_src = b_d[:]
    bias_bcast = bass.AP(
        tensor=bias_src.tensor,
        offset=bias_src.offset,
        ap=[[0, BPC]] + list(bias_src.ap),
    )
    h16_src = hid16_d[:]
    hidR_bcast = bass.AP(
        tensor=h16_src.tensor,
        offset=h16_src.offset,
        ap=[[0, 128]] + list(h16_src.ap),
    )

    def prog_gps():
        g = GPS.eng if GPS.emit else None
        GPS.op(lambda: g.memset(ones_col.ap(), 1.0))
        GPS.op(lambda: g.memset(ones128.ap(), 1.0))
        GPS.op(lambda: g.memset(ident.ap(), 0.0))
        GPS.op(lambda: g.affine_select(
            out=ident.ap(), in_=ident.ap(),
            compare_op=mybir.AluOpType.not_equal, fill=1.0, base=0,
            pattern=[[-1, 128]], channel_multiplier=1))
        GPS.mark("setup")

    def prog_dma():
        d = DMA.eng if DMA.emit else None
        DMA.op(lambda: d.dma_start(out=hid, in_=hid_d[:]), inc=16, sem="hid")
        DMA.mark("hid", sem="hid")
        DMA.op(lambda: d.dma_start(out=bias, in_=bias_bcast), inc=16, sem="bias")
        DMA.mark("bias", sem="bias")
        DMA.op(lambda: d.dma_start(out=hidR.ap(), in_=hidR_bcast), inc=16, sem="hbc")
        DMA.mark("hbc", sem="hbc")
        for b in range(BPC):
            if b >= NL:
                DMA.wait(("pe", "e2", b - NL))
            src = lstm_d[b]
            for q in range(NQ):
                DMA.op(lambda src=src, b=b, q=q: d.dma_start(
                    out=L[b % NL].ap()[:, TQ * q:TQ * (q + 1), :],
                    in_=src[:, TQ * q:TQ * (q + 1), :]),
                    inc=16, sem=f"l{b % NL}")
                DMA.mark("Lq", b, q, sem=f"l{b % NL}")
            DMA.mark("L", b, sem=f"l{b % NL}")
            if b == 2:
                DMA.op(lambda: d.dma_start(out=WT.ap(), in_=wt_d[:]),
                       inc=16, sem="wt")
                DMA.mark("wt", sem="wt")
        DMA.wait(("dve", "bias_hi"))
        DMA.op(lambda: d.dma_start(out=out_d[:], in_=out_sb), inc=16, sem="outd")
        DMA.wait(("outd", counts.get("outd", 0)))

    def prog_pe():
        p = PE.eng if PE.emit else None
        PE.wait(("gps", "setup"))
        PE.wait(("dma", "hid"))
        # hidden^T -> CT chunks 0..7 staging (psum)
        for c in range(HCH):
            PE.op(lambda c=c: p.transpose(
                ctcols8[:, c, :], hid[0:BPC, c * 128:(c + 1) * 128],
                ident.ap()[0:BPC, 0:BPC]))
        PE.mark("hidT")
        PE.wait(("dve", "rmax", 0))
        PE.op(lambda: p.transpose(mpT, mp.ap(), ident.ap()))
        PE.mark("transp", 0)
        for b in range(BPC):
            PE.wait(("dve", "rmax2", b))
            PE.op(lambda b=b: p.matmul(
                negM_bc, lhsT=ones_col.ap(), rhs=negM1[b % 2],
                start=True, stop=True))
            PE.mark("bcast", b)
            if b >= 1:
                PE.wait(("dve", "recip", b - 1))   # Zps slot reuse
            PE.wait(("act", "exp", b))
            PE.op(lambda b=b: p.matmul(
                Zps, lhsT=zp[b % 2].ap(), rhs=ones128.ap(),
                start=True, stop=True))
            PE.mark("z", b)
            # einsum2: attn_unnorm = sum_s w[s] * L[s, :]
            if b >= 1:
                PE.wait(("act", "cphi", b - 1))    # acc bank reuse
            for t in range(T):
                PE.op(lambda b=b, t=t: p.matmul(
                    acc_lo.ap()[0:1, :],
                    lhsT=wexp[b % 2].ap()[:, t:t + 1],
                    rhs=L[b % NL].ap()[:, t, 0:512],
                    start=(t == 0), stop=(t == T - 1)))
                PE.op(lambda b=b, t=t: p.matmul(
                    acc_hi.ap()[0:1, :],
                    lhsT=wexp[b % 2].ap()[:, t:t + 1],
                    rhs=L[b % NL].ap()[:, t, 512:1024],
                    start=(t == 0), stop=(t == T - 1)))
            PE.mark("e2", b)
            if b + 1 < BPC:
                PE.wait(("dve", "rmax", b + 1))
                PE.op(lambda: p.transpose(mpT, mp.ap(), ident.ap()))
                PE.mark("transp", b + 1)
            # attn row -> columns (chunk transposes via K=1 matmuls)
            PE.wait(("act", "cphi", b))
            if b >= 1:
                PE.wait(("act", "ctcp", b - 1))
            for c in range(HCH):
                PE.op(lambda b=b, c=c: p.transpose(
                    ctcols[:, c:c + 1],
                    attn_sb[b % 2][0:1, c * 128:(c + 1) * 128],
                    ones_col.ap()[0:1, 0:1]))
            PE.mark("attnT", b)
        # final projection (fp16): combined^T @ W^T chunks
        PE.wait(("act", "ctcp", BPC - 1))
        PE.wait(("dma", "wt"))
        for c in range(NCH):
            PE.op(lambda c=c: p.matmul(
                acc_lo.ap()[0:BPC, :],
                lhsT=CT.ap()[:, c, :],
                rhs=WT.ap()[:, c, 0:512],
                start=(c == 0), stop=(c == NCH - 1)))
            PE.op(lambda c=c: p.matmul(
                acc_hi.ap()[0:BPC, :],
                lhsT=CT.ap()[:, c, :],
                rhs=WT.ap()[:, c, 512:1024],
                start=(c == 0), stop=(c == NCH - 1)))
        PE.mark("final")

    def prog_dve():
        v = DVE.eng if DVE.emit else None
        # CT hidden columns: psum staging -> CT (f32 -> fp16 convert)
        DVE.wait(("pe", "hidT"))
        DVE.op(lambda: v.tensor_copy(CT.ap()[:, 0:HCH, :], ctcols8))
        DVE.mark("cth")
        for b in range(BPC):
            DVE.wait(("dma", "hbc"))
            for q in range(NQ):
                DVE.wait(("dma", "Lq", b, q))
                if b >= 1 and 4 * q < NACT:
                    # prodQ[q] reuse: last ACT col of this quad, prev batch
                    DVE.wait(("act", "acc", b - 1, min(4 * q + 3, NACT - 1)))
                DVE.op(lambda b=b, q=q: v.tensor_mul(
                    prodQ[q].ap(),
                    L[b % NL].ap()[:, TQ * q:TQ * (q + 1), :],
                    hidR.ap()[:, b:b + 1, :].broadcast_to((128, TQ, H))),
                    drain=False)
                DVE.mark("multq", b, q)
                if q == 1 and b >= 1:
                    DVE.wait(("pe", "z", b - 1))
                    DVE.op(lambda b=b: v.reciprocal(rZ[(b - 1) % 2], Zps),
                           drain=False)
                    DVE.mark("recip", b - 1)
            if b >= 2:
                DVE.wait(("act", "exp", b - 2))   # scores slot reuse
            # DVE-owned score columns NACT..15
            t0 = NACT
            if t0 % 4 != 0:
                # partial quad: cols t0..(next multiple of 4 - 1)
                hi = 4 * ((t0 + 3) // 4)
                DVE.op(lambda t0=t0, hi=hi, b=b: v.tensor_reduce(
                    scores[b % 2].ap()[:, t0:hi],
                    prodQ[t0 // 4].ap()[:, t0 % 4:, :],
                    axis=mybir.AxisListType.X, op=mybir.AluOpType.add))
                t0 = hi
            while t0 < T:
                DVE.op(lambda t0=t0, b=b: v.tensor_reduce(
                    scores[b % 2].ap()[:, t0:t0 + 4],
                    prodQ[t0 // 4].ap(),
                    axis=mybir.AxisListType.X, op=mybir.AluOpType.add),
                    drain=(t0 == NACT))
                t0 += 4
            if b >= 1:
                DVE.wait(("pe", "transp", b - 1))   # mp slot reuse
            DVE.wait(("act", "acc", b, NACT - 1))
            DVE.op(lambda b=b: v.reduce_max(
                mp.ap(), scores[b % 2].ap(), axis=mybir.AxisListType.X))
            DVE.mark("rmax", b)
            DVE.wait(("pe", "transp", b))
            DVE.op(lambda b=b: v.reduce_max(
                negM1[b % 2], mpT, axis=mybir.AxisListType.X, negate=True))
            DVE.mark("rmax2", b)
        DVE.wait(("pe", "z", BPC - 1))
        DVE.op(lambda: v.reciprocal(rZ[(BPC - 1) % 2], Zps))
        DVE.mark("recip", BPC - 1)
        # final bias adds
        DVE.wait(("pe", "final"))
        DVE.op(lambda: v.tensor_add(
            out_sb[:, 0:512], acc_lo.ap()[0:BPC, :], bias[:, 0:512]))
        DVE.mark("bias_lo")
        DVE.wait(("dma", "bias"))
        DVE.op(lambda: v.tensor_add(
            out_sb[:, 512:1024], acc_hi.ap()[0:BPC, :], bias[:, 512:1024]))
        DVE.mark("bias_hi")

    def prog_act():
        a = ACT.eng if ACT.emit else None
        Copy = mybir.ActivationFunctionType.Copy
        Exp = mybir.ActivationFunctionType.Exp
        for b in range(BPC):
            for t in range(NACT):
                ACT.wait(("dve", "multq", b, t // 4))
                ACT.op(lambda b=b, t=t: a.activation(
                    out=dmy.ap()[:, t:t + 1].broadcast_to((128, H)),
                    in_=prodQ[t // 4].ap()[:, t % 4, :],
                    func=Copy, accum_out=scores[b % 2].ap()[:, t:t + 1]),
                    drain=(t == 0))
                ACT.mark("acc", b, t)
            if b >= 2:
                ACT.wait(("pe", "attnT", b - 2))
                ACT.op(lambda b=b: a.activation(
                    out=CT.ap()[:, HCH:NCH, b - 2], in_=ctcols, func=Copy))
                ACT.mark("ctcp", b - 2)
            # previous batch's attn copies -- placed before negMcp/exp so
            # attnT(b-1) -> cphi(b-1) never chains through exp(b)
            if b >= 1:
                ACT.wait(("pe", "e2", b - 1))
                ACT.wait(("dve", "recip", b - 1))
                ACT.op(lambda b=b: a.activation(
                    out=attn_sb[(b - 1) % 2][0:1, 0:512], in_=acc_lo.ap()[0:1, :],
                    func=Copy, scale=rZ[(b - 1) % 2]))
                ACT.mark("cplo", b - 1)
                ACT.op(lambda b=b: a.activation(
                    out=attn_sb[(b - 1) % 2][0:1, 512:1024], in_=acc_hi.ap()[0:1, :],
                    func=Copy, scale=rZ[(b - 1) % 2]))
                ACT.mark("cphi", b - 1)
            ACT.wait(("pe", "bcast", b))
            ACT.op(lambda b=b: a.activation(
                out=negM[b % 2].ap(), in_=negM_bc, func=Copy))
            ACT.mark("negMcp", b)
            if b >= 2:
                ACT.wait(("pe", "e2", b - 2))    # wexp/zp slot reuse
            ACT.op(lambda b=b: a.activation(
                out=wexp[b % 2].ap(), in_=scores[b % 2].ap(), func=Exp,
                bias=negM[b % 2].ap(), scale=1.0, accum_out=zp[b % 2].ap()))
            ACT.mark("exp", b)
        for b in (BPC - 1,):
            ACT.wait(("pe", "e2", b))
            ACT.wait(("dve", "recip", b))
            ACT.op(lambda b=b: a.activation(
                out=attn_sb[b % 2][0:1, 0:512], in_=acc_lo.ap()[0:1, :],
                func=Copy, scale=rZ[b % 2]))
            ACT.mark("cplo", b)
            ACT.op(lambda b=b: a.activation(
                out=attn_sb[b % 2][0:1, 512:1024], in_=acc_hi.ap()[0:1, :],
                func=Copy, scale=rZ[b % 2]))
            ACT.mark("cphi", b)
        for b in (BPC - 2, BPC - 1):
            ACT.wait(("pe", "attnT", b))
            ACT.op(lambda b=b: a.activation(
                out=CT.ap()[:, HCH:NCH, b], in_=ctcols, func=Copy))
            ACT.mark("ctcp", b)

    progs = [
        (GPS, prog_gps), (DMA, prog_dma), (PE, prog_pe),
        (DVE, prog_dve), (ACT, prog_act),
    ]

    # pass 1: count
    for pr, fn in progs:
        pr.begin(emit=False)
        fn()

    # pass 2: emit
    counts.clear()
    sem_names = ["pe", "dve", "act", "gps", "hid", "hbc", "bias",
                 "l0", "l1", "l2", "wt", "outd"]
    with nc.Block() as block:
        for sn in sem_names:
            sems[sn] = nc.alloc_semaphore(name=f"{sn}_sem")

        @block.gpsimd
        def _(eng):
            GPS.begin(eng=eng, emit=True)
            prog_gps()

        @block.sync
        def _(eng):
            DMA.begin(eng=eng, emit=True)
            prog_dma()

        @block.tensor
        def _(eng):
            PE.begin(eng=eng, emit=True)
            prog_pe()

        @block.vector
        def _(eng):
            DVE.begin(eng=eng, emit=True)
            prog_dve()

        @block.scalar
        def _(eng):
            ACT.begin(eng=eng, emit=True)
            prog_act()

    return nc


def kernel(lstm_output, hidden, W_combine, b_combine):
    global _cached_nc, last_results
    lstm_output = np.asarray(lstm_output, dtype=np.float32)
    hidden = np.asarray(hidden, dtype=np.float32)
    W_combine = np.asarray(W_combine, dtype=np.float32)
    b_combine = np.asarray(b_combine, dtype=np.float32)

    if _cached_nc is None:
        _cached_nc = _build_program()
    nc = _cached_nc

    # fp16 partition-major relayout: [B, S, H] -> [B, 128, T, H]
    l16 = lstm_output.astype(np.float16).reshape(B, T, 128, H).transpose(0, 2, 1, 3)
    wt16 = np.ascontiguousarray(
        W_combine.T.astype(np.float16).reshape(NCH, 128, H).transpose(1, 0, 2))
    hid16 = hidden.astype(np.float16)

    in_maps = []
    for i in range(NCORES):
        sl = slice(i * BPC, (i + 1) * BPC)
        in_maps.append({
            "lstm16": np.ascontiguousarray(l16[sl]),
            "hidden": np.ascontiguousarray(hidden[sl]),
            "hidden16": np.ascontiguousarray(hid16[sl]),
            "w_t16": wt16,
            "b_combine": b_combine,
        })
    res = run_bass_kernel_spmd(nc, in_maps, core_ids=list(range(NCORES)))
    last_results = res
    return np.concatenate([res.results[i]["out"] for i in range(NCORES)], axis=0)
